# revision 21
# baseline (speedup 1.0000x reference)
"""Trainium2 Bass kernel: separable 25-tap Gaussian blur (sigma=4) on
[1, 3, 4096, 4096] f32 with edge-replicate padding.

reference computes  blur(img/img.max()) * img.max();  conv is linear, so this
equals blur(img) up to f32 rounding -- the global max is skipped.

Scheme (per core, H sharded 8 ways into 512-row slabs + 12-row halos):
  * host: edge-pad each slab to [3, 536, 4120] fp16 and repack p-major
    ([C,128,4,w] row-tiles + 24-row tail) so every input DMA descriptor is
    one large contiguous block per partition; w-split in two so the
    vertical pass can start before the whole channel lands.
  * vertical pass: fused conv+transpose matmuls.  For each 128-wide
    w-slice j, out_V[w, h] = sum_t X_t[:, wsl].T @ M_t (PSUM accumulation
    over 5 row-tile windows with banded fp16 constant matrices).  Two
    w-slices share a 2-bank PSUM tile; DVE evacuates 1024-wide to fp16.
  * horizontal pass: same structure on Ys (contraction over w), which
    transposes back to natural [h, w] layout; ACT evacuates, DMA out fp16
    with two output rows packed per 16 KB descriptor.
  * tuned against the TRN2 clock governor: the PE only sustains 2.4 GHz
    when at most two compute engines are hot, so evacuation is strictly
    phase-disjoint (DVE during vertical, ACT during horizontal), and 150
    warm-up matmuls during the input-DMA head bring the PE out of its
    low-power state before real work arrives.
"""

import json
import sys

import ml_dtypes
import numpy as np

SIGMA = 4.0
HALF = 12
KSZ = 25
H, W, C = 4096, 4096, 3
N_CORES = 8
SLAB = H // N_CORES          # 512 output rows per core
PAD_W = W + 2 * HALF         # 4120
IN_ROWS = SLAB + 2 * HALF    # 536 input rows per core
N_WTILES = 33                # ceil(4120 / 128); last tile 24 wide
WINDOWS = [(0, 128), (104, 256), (232, 384), (360, 512), (488, 512)]
IN_SCALE = 1.0               # fp16 input needs no scaling
E3 = ml_dtypes.float8_e3m4
OUT_DT_NP = np.float16       # output staged in fp16, upcast on host

_PATCHED = False
_NC_CACHE = {}


def _patch_bass_for_this_walrus():
    """This container's walrus encodes at most ONE inline sem wait per
    instruction ("Too many sync wait commands" otherwise).  Tile freely puts
    several waits on one instruction, so rewrite the BIR JSON at serialization
    time: hoist every multi-wait into standalone EventSemaphore instructions
    (the encoding `wait_ge` uses, which this walrus accepts) placed just
    before the instruction on the same engine queue."""
    global _PATCHED
    if _PATCHED:
        return
    import concourse.bass as bass

    orig = bass.Bass.to_json_bytes

    def _split_multi_waits(self):
        raw = orig(self)
        bir = json.loads(raw)
        ctr = 0
        changed = False
        for fn in bir.get("functions", []):
            for blk in fn.get("blocks", []):
                insts = blk.get("instructions")
                if not insts:
                    continue
                new = []
                for ins in insts:
                    si = ins.get("sync_info")
                    waits = (si or {}).get("on_wait") or []
                    if len(waits) > 1:
                        changed = True
                        for w in waits:
                            ctr += 1
                            ev = {
                                "engine": ins["engine"],
                                "ins": [],
                                "outs": [],
                                "name": f"mwsplit_{ctr}_{ins.get('name', '')}",
                                "opcode": "EventSemaphore",
                                "sync_info": {"on_update": [], "on_wait": [w]},
                            }
                            if "debug" in ins:
                                ev["debug"] = ins["debug"]
                            new.append(ev)
                        si["on_wait"] = []
                    new.append(ins)
                blk["instructions"] = new
        if not changed:
            return raw
        return json.dumps(bir).encode()

    bass.Bass.to_json_bytes = _split_multi_waits
    _PATCHED = True


def _gauss_1d():
    x = np.arange(-HALF, HALF + 1, dtype=np.float64)
    k = np.exp(-0.5 * (x / SIGMA) ** 2)
    return k / k.sum()


def _band_matrices(scale=1.0, dtype=np.float16):
    k = _gauss_1d() * scale
    mf = np.zeros((128, 128), np.float64)
    for p in range(128):
        for n in range(max(0, p - 24), p + 1):
            mf[p, n] = k[p - n]
    mm = np.zeros((128, 152), np.float64)
    for p in range(128):
        for n in range(p, min(152, p + 25)):
            mm[p, n] = k[p - n + 24]
    ml = np.zeros((24, 24), np.float64)
    for p in range(24):
        for n in range(p, 24):
            ml[p, n] = k[p - n + 24]
    return mf.astype(dtype), mm.astype(dtype), ml.astype(dtype)


def _build_nc():
    """Build the per-core SPMD Bass program (all 8 cores run the same code on
    different slabs)."""
    _patch_bass_for_this_walrus()
    import concourse.bass as bass
    import concourse.tile as tile
    from concourse import mybir
    from contextlib import ExitStack

    f8 = mybir.dt.float8e3
    f16 = mybir.dt.float16
    f32 = mybir.dt.float32
    out_dt = f16 if OUT_DT_NP == np.float16 else f32

    # band matrices; the horizontal set carries 1/IN_SCALE.  Packed into one
    # [128, 608] fp16 block (cols: mf 128 | mm 152 | ml 24 | x2) so startup
    # is a single small DMA.
    mfv_np, mmv_np, mlv_np = _band_matrices(1.0, np.float16)
    mfh_np, mmh_np, mlh_np = _band_matrices(1.0 / IN_SCALE, np.float16)
    packed = np.zeros((128, 608), np.float16)
    for off, (mf_, mm_, ml_) in ((0, (mfv_np, mmv_np, mlv_np)),
                                 (304, (mfh_np, mmh_np, mlh_np))):
        packed[:, off : off + 128] = mf_
        packed[:, off + 128 : off + 280] = mm_
        packed[0:24, off + 280 : off + 304] = ml_

    nc = bass.Bass()
    WSPL = 2176                  # w split point for the input DMA halves
    x1a = nc.declare_dram_parameter("x1a", [C, 128, 4, WSPL], f16, isOutput=False)
    x1b = nc.declare_dram_parameter(
        "x1b", [C, 128, 4, PAD_W - WSPL], f16, isOutput=False
    )
    x2 = nc.declare_dram_parameter("x2", [C, 24, PAD_W], f16, isOutput=False)
    y = nc.declare_dram_parameter("y", [C, 2, 128, 2, W], out_dt, isOutput=True)
    packed_d = nc.inline_tensor(packed, name="bands")

    with tile.TileContext(nc) as tc, ExitStack() as ctx:
        consts = ctx.enter_context(tc.tile_pool(name="consts", bufs=1))
        xpool = ctx.enter_context(tc.tile_pool(name="xp", bufs=2))
        yspool = ctx.enter_context(tc.tile_pool(name="ys", bufs=2))
        opool = ctx.enter_context(tc.tile_pool(name="ostage", bufs=2))
        psv = ctx.enter_context(tc.tile_pool(name="psv", bufs=2, space="PSUM"))
        psh = ctx.enter_context(tc.tile_pool(name="psh", bufs=2, space="PSUM"))

        bands = consts.tile([128, 608], f16)
        nc.sync.dma_start(bands[:], packed_d[:])
        mats_v = [bands[:, 0:128], bands[:, 128:280], bands[:, 128:280],
                  bands[:, 128:280], bands[0:24, 280:304]]
        mats_h = [bands[:, 304:432], bands[:, 432:584], bands[:, 432:584],
                  bands[:, 432:584], bands[0:24, 584:608]]

        # pre-warm the tensor engine's clock governor while the first
        # channel's input DMA is in flight: harmless matmuls on the const tile
        wv = psv.tile([128, 1024], f32, name="pv")
        for _ in range(150):
            nc.tensor.matmul(
                out=wv[:, 0:128], lhsT=bands[:, 0:128], rhs=bands[:, 0:128],
                start=True, stop=True,
            )

        for c in range(C):
            xt = xpool.tile([128, 5, PAD_W], f16)
            # p-major packed contiguous descriptors; w-split so the first
            # half of the vertical pass can start before the rest lands
            nc.sync.dma_start(xt[0:24, 4, :], x2[c])
            nc.sync.dma_start(xt[:, 0:4, 0:WSPL], x1a[c])
            nc.sync.dma_start(xt[:, 0:4, WSPL:PAD_W], x1b[c])

            ys = yspool.tile([128, N_WTILES, 512], f16)

            # vertical pass (conv over h, output transposed to [w, h]);
            # two w-slices share a 2-bank PSUM tile -> 1024-wide DVE evacs
            for jp in range((N_WTILES + 1) // 2):
                js = [2 * jp] + ([2 * jp + 1] if 2 * jp + 1 < N_WTILES else [])
                pv = psv.tile([128, 1024], f32, name="pv")
                for ji, j in enumerate(js):
                    m = 128 if j < N_WTILES - 1 else PAD_W - 128 * (N_WTILES - 1)
                    for t in range(5):
                        n0, n1 = WINDOWS[t]
                        kp = 128 if t < 4 else 24
                        nc.tensor.matmul(
                            out=pv[0:m, 512 * ji + n0 : 512 * ji + n1],
                            lhsT=xt[0:kp, t, 128 * j : 128 * j + m],
                            rhs=mats_v[t][0:kp, 0 : n1 - n0],
                            start=(t == 0),
                            stop=(t == 4),
                        )
                vcopy = nc.vector.tensor_copy
                if len(js) == 2:
                    vcopy(ys[:, js[0] : js[0] + 2, :], pv[:, :])
                else:
                    m = PAD_W - 128 * (N_WTILES - 1)
                    vcopy(ys[0:m, js[0], :], pv[0:m, 0:512])

            # horizontal pass (conv over w, transposes back to [h, w]);
            # two h-blocks share one staging tile so each output DMA
            # descriptor covers two DRAM rows (16 KB contiguous)
            for b2 in range(2):
                ot = opool.tile([128, 2, W], out_dt)
                for bi in range(2):
                    b = 2 * b2 + bi
                    for qp in range(W // 1024):
                        ph = psh.tile([128, 1024], f32)
                        for qi in range(2):
                            q = 2 * qp + qi
                            for t in range(5):
                                j = 4 * q + t
                                n0, n1 = WINDOWS[t]
                                kp = 128 if (t < 4 and j < N_WTILES - 1) else 24
                                nc.tensor.matmul(
                                    out=ph[:, 512 * qi + n0 : 512 * qi + n1],
                                    lhsT=ys[0:kp, j, 128 * b : 128 * b + 128],
                                    rhs=mats_h[t][0:kp, 0 : n1 - n0],
                                    start=(t == 0),
                                    stop=(t == 4),
                                )
                        nc.scalar.copy(
                            ot[:, bi, 1024 * qp : 1024 * qp + 1024], ph[:, :]
                        )
                if c == C - 1 and b2 == 1:
                    # last output: split per h-block so the first half's DMA
                    # overlaps the second half's evacuation
                    nc.sync.dma_start(y[c, b2, :, 0:1, :], ot[:, 0:1, :])
                    nc.sync.dma_start(y[c, b2, :, 1:2, :], ot[:, 1:2, :])
                else:
                    nc.sync.dma_start(y[c, b2], ot[:])
    return nc


def _get_nc():
    if "nc" not in _NC_CACHE:
        _NC_CACHE["nc"] = _build_nc()
    return _NC_CACHE["nc"]


def _shard_inputs(img):
    """img [1,3,4096,4096] f32 -> per-core packed fp16 slabs.

    x1 [C,128,4,PAD_W]: x1[c,p,t,:] = padded row 128*t+p of the slab (one
    contiguous 33 KB DMA descriptor per partition).  x2 [C,24,PAD_W]: the
    24 tail rows."""
    x = np.asarray(img)[0]
    xp = np.pad(
        x.astype(np.float16), ((0, 0), (HALF, HALF), (HALF, HALF)), mode="edge"
    )
    in_maps = []
    for core in range(N_CORES):
        sl = xp[:, SLAB * core : SLAB * core + IN_ROWS]      # [3, 536, 4120]
        x1 = sl[:, 0:512].reshape(C, 4, 128, PAD_W).transpose(0, 2, 1, 3)
        x1a = np.ascontiguousarray(x1[:, :, :, 0:2176])
        x1b = np.ascontiguousarray(x1[:, :, :, 2176:PAD_W])
        x2 = np.ascontiguousarray(sl[:, 512:IN_ROWS])
        in_maps.append({"x1a": x1a, "x1b": x1b, "x2": x2})
    return in_maps


def kernel(img):
    import os

    # a stale low-clock device state (seen after wedges) costs ~18%; a core
    # reset at open restores the full 2.4 GHz PE clock
    os.environ.setdefault("NEURON_RT_RESET_CORES", "1")
    from concourse.bass_utils import run_bass_kernel_spmd

    nc = _get_nc()
    in_maps = _shard_inputs(img)
    core_ids = list(range(N_CORES))

    trace = bool(os.environ.get("KNN_TRACE"))
    res = run_bass_kernel_spmd(nc, in_maps, core_ids, trace=trace)
    _NC_CACHE["last_exec_time_ns"] = res.exec_time_ns
    _NC_CACHE["last_results"] = res

    out = np.empty((C, H, W), np.float32)
    for core in core_ids:
        yc = res.results[core]["y"]                      # [C, 2, 128, 2, W]
        yc = yc.transpose(0, 1, 3, 2, 4).reshape(C, SLAB, W)
        out[:, SLAB * core : SLAB * (core + 1), :] = yc.astype(np.float32)
    return out


if __name__ == "__main__":
    # native compile smoke (no hardware)
    import tempfile
    from concourse.bass_utils import compile_bass_kernel

    nc = _build_nc()
    with tempfile.TemporaryDirectory() as td:
        neff = compile_bass_kernel(nc, td)
        print("COMPILED OK:", neff)


# revision 22
# speedup vs baseline: 1.1213x; 1.1213x over previous
"""Trainium2 Bass kernel: separable 25-tap Gaussian blur (sigma=4) on
[1, 3, 4096, 4096] f32 with edge-replicate padding.

reference computes  blur(img/img.max()) * img.max();  conv is linear, so this
equals blur(img) up to f32 rounding -- the global max is skipped.

Scheme (per core, H sharded 8 ways into 512-row slabs + 12-row halos):
  * host: edge-pad each slab to [3, 536, 4120] fp16 and repack p-major
    ([C,128,4,w] row-tiles + 24-row tail) so every input DMA descriptor is
    one large contiguous block per partition; w-split in two so the
    vertical pass can start before the whole channel lands.
  * vertical pass: fused conv+transpose matmuls.  For each 128-wide
    w-slice j, out_V[w, h] = sum_t X_t[:, wsl].T @ M_t (PSUM accumulation
    over 5 row-tile windows with banded fp16 constant matrices).  Two
    w-slices share a 2-bank PSUM tile; DVE evacuates 1024-wide to fp16.
  * horizontal pass: same structure on Ys (contraction over w), which
    transposes back to natural [h, w] layout; ACT evacuates, DMA out fp16
    with two output rows packed per 16 KB descriptor.
  * tuned against the TRN2 clock governor: the PE only sustains 2.4 GHz
    when at most two compute engines are hot, so evacuation is strictly
    phase-disjoint (DVE during vertical, ACT during horizontal), and 150
    warm-up matmuls during the input-DMA head bring the PE out of its
    low-power state before real work arrives.
"""

import json
import sys

import ml_dtypes
import numpy as np

SIGMA = 4.0
HALF = 12
KSZ = 25
H, W, C = 4096, 4096, 3
N_CORES = 8
SLAB = H // N_CORES          # 512 output rows per core
PAD_W = W + 2 * HALF         # 4120
IN_ROWS = SLAB + 2 * HALF    # 536 input rows per core
N_WTILES = 33                # ceil(4120 / 128); last tile 24 wide
WINDOWS = [(0, 128), (104, 256), (232, 384), (360, 512), (488, 512)]
IN_SCALE = 1.0               # fp16 input needs no scaling
E3 = ml_dtypes.float8_e3m4
OUT_DT_NP = np.float16       # output staged in fp16, upcast on host

_PATCHED = False
_NC_CACHE = {}


def _patch_bass_for_this_walrus():
    """This container's walrus encodes at most ONE inline sem wait per
    instruction ("Too many sync wait commands" otherwise).  Tile freely puts
    several waits on one instruction, so rewrite the BIR JSON at serialization
    time: hoist every multi-wait into standalone EventSemaphore instructions
    (the encoding `wait_ge` uses, which this walrus accepts) placed just
    before the instruction on the same engine queue."""
    global _PATCHED
    if _PATCHED:
        return
    import concourse.bass as bass

    orig = bass.Bass.to_json_bytes

    def _split_multi_waits(self):
        raw = orig(self)
        bir = json.loads(raw)
        ctr = 0
        changed = False
        for fn in bir.get("functions", []):
            for blk in fn.get("blocks", []):
                insts = blk.get("instructions")
                if not insts:
                    continue
                new = []
                for ins in insts:
                    si = ins.get("sync_info")
                    waits = (si or {}).get("on_wait") or []
                    if len(waits) > 1:
                        changed = True
                        for w in waits:
                            ctr += 1
                            ev = {
                                "engine": ins["engine"],
                                "ins": [],
                                "outs": [],
                                "name": f"mwsplit_{ctr}_{ins.get('name', '')}",
                                "opcode": "EventSemaphore",
                                "sync_info": {"on_update": [], "on_wait": [w]},
                            }
                            if "debug" in ins:
                                ev["debug"] = ins["debug"]
                            new.append(ev)
                        si["on_wait"] = []
                    new.append(ins)
                blk["instructions"] = new
        if not changed:
            return raw
        return json.dumps(bir).encode()

    bass.Bass.to_json_bytes = _split_multi_waits
    _PATCHED = True


def _gauss_1d():
    x = np.arange(-HALF, HALF + 1, dtype=np.float64)
    k = np.exp(-0.5 * (x / SIGMA) ** 2)
    return k / k.sum()


def _band_matrices(scale=1.0, dtype=np.float16):
    k = _gauss_1d() * scale
    mf = np.zeros((128, 128), np.float64)
    for p in range(128):
        for n in range(max(0, p - 24), p + 1):
            mf[p, n] = k[p - n]
    mm = np.zeros((128, 152), np.float64)
    for p in range(128):
        for n in range(p, min(152, p + 25)):
            mm[p, n] = k[p - n + 24]
    ml = np.zeros((24, 24), np.float64)
    for p in range(24):
        for n in range(p, 24):
            ml[p, n] = k[p - n + 24]
    return mf.astype(dtype), mm.astype(dtype), ml.astype(dtype)


def _build_nc():
    """Build the per-core SPMD Bass program (all 8 cores run the same code on
    different slabs)."""
    _patch_bass_for_this_walrus()
    import concourse.bass as bass
    import concourse.tile as tile
    from concourse import mybir
    from contextlib import ExitStack

    f8 = mybir.dt.float8e3
    f16 = mybir.dt.float16
    f32 = mybir.dt.float32
    out_dt = f16 if OUT_DT_NP == np.float16 else f32

    # band matrices; the horizontal set carries 1/IN_SCALE.  Packed into one
    # [128, 608] fp16 block (cols: mf 128 | mm 152 | ml 24 | x2) so startup
    # is a single small DMA.
    mfv_np, mmv_np, mlv_np = _band_matrices(1.0, np.float16)
    mfh_np, mmh_np, mlh_np = _band_matrices(1.0 / IN_SCALE, np.float16)
    packed = np.zeros((128, 608), np.float16)
    for off, (mf_, mm_, ml_) in ((0, (mfv_np, mmv_np, mlv_np)),
                                 (304, (mfh_np, mmh_np, mlh_np))):
        packed[:, off : off + 128] = mf_
        packed[:, off + 128 : off + 280] = mm_
        packed[0:24, off + 280 : off + 304] = ml_

    nc = bass.Bass()
    WSPL = 2176                  # w split point for the input DMA halves
    x1a = nc.declare_dram_parameter("x1a", [C, 128, 4, WSPL], f16, isOutput=False)
    x1b = nc.declare_dram_parameter(
        "x1b", [C, 128, 4, PAD_W - WSPL], f16, isOutput=False
    )
    x2 = nc.declare_dram_parameter("x2", [C, 24, PAD_W], f16, isOutput=False)
    y = nc.declare_dram_parameter("y", [C, 2, 128, 2, W], out_dt, isOutput=True)
    packed_d = nc.inline_tensor(packed, name="bands")

    with tile.TileContext(nc) as tc, ExitStack() as ctx:
        consts = ctx.enter_context(tc.tile_pool(name="consts", bufs=1))
        xpool = ctx.enter_context(tc.tile_pool(name="xp", bufs=2))
        yspool = ctx.enter_context(tc.tile_pool(name="ys", bufs=2))
        opool = ctx.enter_context(tc.tile_pool(name="ostage", bufs=2))
        psv = ctx.enter_context(tc.tile_pool(name="psv", bufs=2, space="PSUM"))
        psh = ctx.enter_context(tc.tile_pool(name="psh", bufs=2, space="PSUM"))

        bands = consts.tile([128, 608], f16)
        nc.sync.dma_start(bands[:], packed_d[:])
        mats_v = [bands[:, 0:128], bands[:, 128:280], bands[:, 128:280],
                  bands[:, 128:280], bands[0:24, 280:304]]
        mats_h = [bands[:, 304:432], bands[:, 432:584], bands[:, 432:584],
                  bands[:, 432:584], bands[0:24, 584:608]]

        # pre-warm the tensor engine's clock governor while the first
        # channel's input DMA is in flight: harmless matmuls on the const tile
        wv = psv.tile([128, 1024], f32, name="pv")
        for _ in range(115):
            nc.tensor.matmul(
                out=wv[:, 0:128], lhsT=bands[:, 0:128], rhs=bands[:, 0:128],
                start=True, stop=True,
            )

        for c in range(C):
            xt = xpool.tile([128, 5, PAD_W], f16)
            # p-major packed contiguous descriptors; w-split so the first
            # half of the vertical pass can start before the rest lands
            nc.sync.dma_start(xt[0:24, 4, :], x2[c])
            nc.sync.dma_start(xt[:, 0:4, 0:WSPL], x1a[c])
            nc.sync.dma_start(xt[:, 0:4, WSPL:PAD_W], x1b[c])

            ys = yspool.tile([128, N_WTILES, 512], f16)

            # vertical pass (conv over h, output transposed to [w, h]);
            # two w-slices share a 2-bank PSUM tile -> 1024-wide DVE evacs
            for jp in range((N_WTILES + 1) // 2):
                js = [2 * jp] + ([2 * jp + 1] if 2 * jp + 1 < N_WTILES else [])
                pv = psv.tile([128, 1024], f32, name="pv")
                for ji, j in enumerate(js):
                    m = 128 if j < N_WTILES - 1 else PAD_W - 128 * (N_WTILES - 1)
                    for t in range(5):
                        n0, n1 = WINDOWS[t]
                        kp = 128 if t < 4 else 24
                        nc.tensor.matmul(
                            out=pv[0:m, 512 * ji + n0 : 512 * ji + n1],
                            lhsT=xt[0:kp, t, 128 * j : 128 * j + m],
                            rhs=mats_v[t][0:kp, 0 : n1 - n0],
                            start=(t == 0),
                            stop=(t == 4),
                        )
                vcopy = nc.vector.tensor_copy
                if len(js) == 2:
                    vcopy(ys[:, js[0] : js[0] + 2, :], pv[:, :])
                else:
                    m = PAD_W - 128 * (N_WTILES - 1)
                    vcopy(ys[0:m, js[0], :], pv[0:m, 0:512])

            # horizontal pass (conv over w, transposes back to [h, w]);
            # two h-blocks share one staging tile so each output DMA
            # descriptor covers two DRAM rows (16 KB contiguous)
            for b2 in range(2):
                ot = opool.tile([128, 2, W], out_dt)
                for bi in range(2):
                    b = 2 * b2 + bi
                    for qp in range(W // 1024):
                        ph = psh.tile([128, 1024], f32)
                        for qi in range(2):
                            q = 2 * qp + qi
                            for t in range(5):
                                j = 4 * q + t
                                n0, n1 = WINDOWS[t]
                                kp = 128 if (t < 4 and j < N_WTILES - 1) else 24
                                nc.tensor.matmul(
                                    out=ph[:, 512 * qi + n0 : 512 * qi + n1],
                                    lhsT=ys[0:kp, j, 128 * b : 128 * b + 128],
                                    rhs=mats_h[t][0:kp, 0 : n1 - n0],
                                    start=(t == 0),
                                    stop=(t == 4),
                                )
                        nc.scalar.copy(
                            ot[:, bi, 1024 * qp : 1024 * qp + 1024], ph[:, :]
                        )
                if c == C - 1 and b2 == 1:
                    # last output: chunk the DMA so it drains behind the
                    # evacuations instead of waiting for all of them
                    for qp in range(4):
                        w0 = 1024 * qp
                        nc.sync.dma_start(
                            y[c, b2, :, :, w0 : w0 + 1024],
                            ot[:, :, w0 : w0 + 1024],
                        )
                else:
                    nc.sync.dma_start(y[c, b2], ot[:])
    return nc


def _get_nc():
    if "nc" not in _NC_CACHE:
        _NC_CACHE["nc"] = _build_nc()
    return _NC_CACHE["nc"]


def _shard_inputs(img):
    """img [1,3,4096,4096] f32 -> per-core packed fp16 slabs.

    x1 [C,128,4,PAD_W]: x1[c,p,t,:] = padded row 128*t+p of the slab (one
    contiguous 33 KB DMA descriptor per partition).  x2 [C,24,PAD_W]: the
    24 tail rows."""
    x = np.asarray(img)[0]
    xp = np.pad(
        x.astype(np.float16), ((0, 0), (HALF, HALF), (HALF, HALF)), mode="edge"
    )
    in_maps = []
    for core in range(N_CORES):
        sl = xp[:, SLAB * core : SLAB * core + IN_ROWS]      # [3, 536, 4120]
        x1 = sl[:, 0:512].reshape(C, 4, 128, PAD_W).transpose(0, 2, 1, 3)
        x1a = np.ascontiguousarray(x1[:, :, :, 0:2176])
        x1b = np.ascontiguousarray(x1[:, :, :, 2176:PAD_W])
        x2 = np.ascontiguousarray(sl[:, 512:IN_ROWS])
        in_maps.append({"x1a": x1a, "x1b": x1b, "x2": x2})
    return in_maps


def kernel(img):
    import os

    # a stale low-clock device state (seen after wedges) costs ~18%; a core
    # reset at open restores the full 2.4 GHz PE clock
    os.environ.setdefault("NEURON_RT_RESET_CORES", "1")
    from concourse.bass_utils import run_bass_kernel_spmd

    nc = _get_nc()
    in_maps = _shard_inputs(img)
    core_ids = list(range(N_CORES))

    trace = bool(os.environ.get("KNN_TRACE"))
    res = run_bass_kernel_spmd(nc, in_maps, core_ids, trace=trace)
    _NC_CACHE["last_exec_time_ns"] = res.exec_time_ns
    _NC_CACHE["last_results"] = res

    out = np.empty((C, H, W), np.float32)
    for core in core_ids:
        yc = res.results[core]["y"]                      # [C, 2, 128, 2, W]
        yc = yc.transpose(0, 1, 3, 2, 4).reshape(C, SLAB, W)
        out[:, SLAB * core : SLAB * (core + 1), :] = yc.astype(np.float32)
    return out


if __name__ == "__main__":
    # native compile smoke (no hardware)
    import tempfile
    from concourse.bass_utils import compile_bass_kernel

    nc = _build_nc()
    with tempfile.TemporaryDirectory() as td:
        neff = compile_bass_kernel(nc, td)
        print("COMPILED OK:", neff)


# revision 23
# speedup vs baseline: 1.2299x; 1.0969x over previous
"""Trainium2 Bass kernel: separable 25-tap Gaussian blur (sigma=4) on
[1, 3, 4096, 4096] f32 with edge-replicate padding.

reference computes  blur(img/img.max()) * img.max();  conv is linear, so this
equals blur(img) up to f32 rounding -- the global max is skipped.

Scheme (per core, H sharded 8 ways into 512-row slabs + 12-row halos):
  * host: edge-pad each slab to [3, 536, 4120] fp16 and repack p-major
    ([C,128,4,w] row-tiles + 24-row tail) so every input DMA descriptor is
    one large contiguous block per partition; w-split in two so the
    vertical pass can start before the whole channel lands.
  * vertical pass: fused conv+transpose matmuls.  For each 128-wide
    w-slice j, out_V[w, h] = sum_t X_t[:, wsl].T @ M_t (PSUM accumulation
    over 5 row-tile windows with banded fp16 constant matrices).  Two
    w-slices share a 2-bank PSUM tile; DVE evacuates 1024-wide to fp16.
  * horizontal pass: same structure on Ys (contraction over w), which
    transposes back to natural [h, w] layout; ACT evacuates, DMA out fp16
    with two output rows packed per 16 KB descriptor.
  * tuned against the TRN2 clock governor: the PE only sustains 2.4 GHz
    when at most two compute engines are hot, so evacuation is strictly
    phase-disjoint (DVE during vertical, ACT during horizontal), and 150
    warm-up matmuls during the input-DMA head bring the PE out of its
    low-power state before real work arrives.
"""

import json
import sys

import ml_dtypes
import numpy as np

SIGMA = 4.0
HALF = 12
KSZ = 25
H, W, C = 4096, 4096, 3
N_CORES = 8
SLAB = H // N_CORES          # 512 output rows per core
PAD_W = W + 2 * HALF         # 4120
IN_ROWS = SLAB + 2 * HALF    # 536 input rows per core
N_WTILES = 33                # ceil(4120 / 128); last tile 24 wide
WINDOWS = [(0, 128), (104, 256), (232, 384), (360, 512), (488, 512)]
IN_SCALE = 1.0               # fp16 input needs no scaling
E3 = ml_dtypes.float8_e3m4
OUT_DT_NP = np.float16       # output staged in fp16, upcast on host

_PATCHED = False
_NC_CACHE = {}


def _patch_bass_for_this_walrus():
    """This container's walrus encodes at most ONE inline sem wait per
    instruction ("Too many sync wait commands" otherwise).  Tile freely puts
    several waits on one instruction, so rewrite the BIR JSON at serialization
    time: hoist every multi-wait into standalone EventSemaphore instructions
    (the encoding `wait_ge` uses, which this walrus accepts) placed just
    before the instruction on the same engine queue."""
    global _PATCHED
    if _PATCHED:
        return
    import concourse.bass as bass

    orig = bass.Bass.to_json_bytes

    def _split_multi_waits(self):
        raw = orig(self)
        bir = json.loads(raw)
        ctr = 0
        changed = False
        for fn in bir.get("functions", []):
            for blk in fn.get("blocks", []):
                insts = blk.get("instructions")
                if not insts:
                    continue
                new = []
                for ins in insts:
                    si = ins.get("sync_info")
                    waits = (si or {}).get("on_wait") or []
                    if len(waits) > 1:
                        changed = True
                        for w in waits:
                            ctr += 1
                            ev = {
                                "engine": ins["engine"],
                                "ins": [],
                                "outs": [],
                                "name": f"mwsplit_{ctr}_{ins.get('name', '')}",
                                "opcode": "EventSemaphore",
                                "sync_info": {"on_update": [], "on_wait": [w]},
                            }
                            if "debug" in ins:
                                ev["debug"] = ins["debug"]
                            new.append(ev)
                        si["on_wait"] = []
                    new.append(ins)
                blk["instructions"] = new
        if not changed:
            return raw
        return json.dumps(bir).encode()

    bass.Bass.to_json_bytes = _split_multi_waits
    _PATCHED = True


def _gauss_1d():
    x = np.arange(-HALF, HALF + 1, dtype=np.float64)
    k = np.exp(-0.5 * (x / SIGMA) ** 2)
    return k / k.sum()


def _band_matrices(scale=1.0, dtype=np.float16):
    k = _gauss_1d() * scale
    mf = np.zeros((128, 128), np.float64)
    for p in range(128):
        for n in range(max(0, p - 24), p + 1):
            mf[p, n] = k[p - n]
    mm = np.zeros((128, 152), np.float64)
    for p in range(128):
        for n in range(p, min(152, p + 25)):
            mm[p, n] = k[p - n + 24]
    ml = np.zeros((24, 24), np.float64)
    for p in range(24):
        for n in range(p, 24):
            ml[p, n] = k[p - n + 24]
    return mf.astype(dtype), mm.astype(dtype), ml.astype(dtype)


def _build_nc():
    """Build the per-core SPMD Bass program (all 8 cores run the same code on
    different slabs)."""
    _patch_bass_for_this_walrus()
    import concourse.bass as bass
    import concourse.tile as tile
    from concourse import mybir
    from contextlib import ExitStack

    f8 = mybir.dt.float8e3
    f16 = mybir.dt.float16
    f32 = mybir.dt.float32
    out_dt = f16 if OUT_DT_NP == np.float16 else f32

    # band matrices; the horizontal set carries 1/IN_SCALE.  Packed into one
    # [128, 608] fp16 block (cols: mf 128 | mm 152 | ml 24 | x2) so startup
    # is a single small DMA.
    mfv_np, mmv_np, mlv_np = _band_matrices(1.0, np.float16)
    mfh_np, mmh_np, mlh_np = _band_matrices(1.0 / IN_SCALE, np.float16)
    packed = np.zeros((128, 608), np.float16)
    for off, (mf_, mm_, ml_) in ((0, (mfv_np, mmv_np, mlv_np)),
                                 (304, (mfh_np, mmh_np, mlh_np))):
        packed[:, off : off + 128] = mf_
        packed[:, off + 128 : off + 280] = mm_
        packed[0:24, off + 280 : off + 304] = ml_

    nc = bass.Bass()
    WSPL = 2176                  # w split point for the input DMA halves
    x1a = nc.declare_dram_parameter("x1a", [C, 128, 4, WSPL], f16, isOutput=False)
    x1b = nc.declare_dram_parameter(
        "x1b", [C, 128, 4, PAD_W - WSPL], f16, isOutput=False
    )
    x2 = nc.declare_dram_parameter("x2", [C, 24, PAD_W], f16, isOutput=False)
    y = nc.declare_dram_parameter("y", [C, 2, 128, 2, W], out_dt, isOutput=True)
    packed_d = nc.inline_tensor(packed, name="bands")

    with tile.TileContext(nc) as tc, ExitStack() as ctx:
        consts = ctx.enter_context(tc.tile_pool(name="consts", bufs=1))
        xpool = ctx.enter_context(tc.tile_pool(name="xp", bufs=2))
        yspool = ctx.enter_context(tc.tile_pool(name="ys", bufs=2))
        opool = ctx.enter_context(tc.tile_pool(name="ostage", bufs=2))
        psv = ctx.enter_context(tc.tile_pool(name="psv", bufs=2, space="PSUM"))
        psh = ctx.enter_context(tc.tile_pool(name="psh", bufs=2, space="PSUM"))

        bands = consts.tile([128, 608], f16)
        nc.sync.dma_start(bands[:], packed_d[:])
        mats_v = [bands[:, 0:128], bands[:, 128:280], bands[:, 128:280],
                  bands[:, 128:280], bands[0:24, 280:304]]
        mats_h = [bands[:, 304:432], bands[:, 432:584], bands[:, 432:584],
                  bands[:, 432:584], bands[0:24, 584:608]]

        # pre-warm the tensor engine's clock governor while the first
        # channel's input DMA is in flight: harmless matmuls on the const tile
        wv = psv.tile([128, 1024], f32, name="pv")
        for _ in range(150):
            nc.tensor.matmul(
                out=wv[:, 0:128], lhsT=bands[:, 0:128], rhs=bands[:, 0:128],
                start=True, stop=True,
            )

        for c in range(C):
            xt = xpool.tile([128, 5, PAD_W], f16)
            # p-major packed contiguous descriptors; w-split so the first
            # half of the vertical pass can start before the rest lands
            nc.sync.dma_start(xt[0:24, 4, :], x2[c])
            nc.sync.dma_start(xt[:, 0:4, 0:WSPL], x1a[c])
            nc.sync.dma_start(xt[:, 0:4, WSPL:PAD_W], x1b[c])

            ys = yspool.tile([128, N_WTILES, 512], f16)

            # vertical pass (conv over h, output transposed to [w, h]);
            # two w-slices share a 2-bank PSUM tile -> 1024-wide DVE evacs
            for jp in range((N_WTILES + 1) // 2):
                js = [2 * jp] + ([2 * jp + 1] if 2 * jp + 1 < N_WTILES else [])
                pv = psv.tile([128, 1024], f32, name="pv")
                for ji, j in enumerate(js):
                    m = 128 if j < N_WTILES - 1 else PAD_W - 128 * (N_WTILES - 1)
                    for t in range(5):
                        n0, n1 = WINDOWS[t]
                        kp = 128 if t < 4 else 24
                        nc.tensor.matmul(
                            out=pv[0:m, 512 * ji + n0 : 512 * ji + n1],
                            lhsT=xt[0:kp, t, 128 * j : 128 * j + m],
                            rhs=mats_v[t][0:kp, 0 : n1 - n0],
                            start=(t == 0),
                            stop=(t == 4),
                        )
                vcopy = nc.vector.tensor_copy
                if len(js) == 2:
                    vcopy(ys[:, js[0] : js[0] + 2, :], pv[:, :])
                else:
                    m = PAD_W - 128 * (N_WTILES - 1)
                    vcopy(ys[0:m, js[0], :], pv[0:m, 0:512])

            # horizontal pass (conv over w, transposes back to [h, w]);
            # two h-blocks share one staging tile so each output DMA
            # descriptor covers two DRAM rows (16 KB contiguous)
            for b2 in range(2):
                ot = opool.tile([128, 2, W], out_dt)
                for bi in range(2):
                    b = 2 * b2 + bi
                    for qp in range(W // 1024):
                        ph = psh.tile([128, 1024], f32)
                        for qi in range(2):
                            q = 2 * qp + qi
                            for t in range(5):
                                j = 4 * q + t
                                n0, n1 = WINDOWS[t]
                                kp = 128 if (t < 4 and j < N_WTILES - 1) else 24
                                nc.tensor.matmul(
                                    out=ph[:, 512 * qi + n0 : 512 * qi + n1],
                                    lhsT=ys[0:kp, j, 128 * b : 128 * b + 128],
                                    rhs=mats_h[t][0:kp, 0 : n1 - n0],
                                    start=(t == 0),
                                    stop=(t == 4),
                                )
                        nc.scalar.copy(
                            ot[:, bi, 1024 * qp : 1024 * qp + 1024], ph[:, :]
                        )
                if c == C - 1 and b2 == 1:
                    # last output: split per h-block so the first half's DMA
                    # overlaps the second half's evacuation
                    nc.sync.dma_start(y[c, b2, :, 0:1, :], ot[:, 0:1, :])
                    nc.sync.dma_start(y[c, b2, :, 1:2, :], ot[:, 1:2, :])
                else:
                    nc.sync.dma_start(y[c, b2], ot[:])
    return nc


def _get_nc():
    if "nc" not in _NC_CACHE:
        _NC_CACHE["nc"] = _build_nc()
    return _NC_CACHE["nc"]


def _shard_inputs(img):
    """img [1,3,4096,4096] f32 -> per-core packed fp16 slabs.

    x1 [C,128,4,PAD_W]: x1[c,p,t,:] = padded row 128*t+p of the slab (one
    contiguous 33 KB DMA descriptor per partition).  x2 [C,24,PAD_W]: the
    24 tail rows."""
    x = np.asarray(img)[0]
    xp = np.pad(
        x.astype(np.float16), ((0, 0), (HALF, HALF), (HALF, HALF)), mode="edge"
    )
    in_maps = []
    for core in range(N_CORES):
        sl = xp[:, SLAB * core : SLAB * core + IN_ROWS]      # [3, 536, 4120]
        x1 = sl[:, 0:512].reshape(C, 4, 128, PAD_W).transpose(0, 2, 1, 3)
        x1a = np.ascontiguousarray(x1[:, :, :, 0:2176])
        x1b = np.ascontiguousarray(x1[:, :, :, 2176:PAD_W])
        x2 = np.ascontiguousarray(sl[:, 512:IN_ROWS])
        in_maps.append({"x1a": x1a, "x1b": x1b, "x2": x2})
    return in_maps


def kernel(img):
    import os

    # a stale low-clock device state (seen after wedges) costs ~18%; a core
    # reset at open restores the full 2.4 GHz PE clock
    os.environ.setdefault("NEURON_RT_RESET_CORES", "1")
    from concourse.bass_utils import run_bass_kernel_spmd

    nc = _get_nc()
    in_maps = _shard_inputs(img)
    core_ids = list(range(N_CORES))

    trace = bool(os.environ.get("KNN_TRACE"))
    res = run_bass_kernel_spmd(nc, in_maps, core_ids, trace=trace)
    _NC_CACHE["last_exec_time_ns"] = res.exec_time_ns
    _NC_CACHE["last_results"] = res

    out = np.empty((C, H, W), np.float32)
    for core in core_ids:
        yc = res.results[core]["y"]                      # [C, 2, 128, 2, W]
        yc = yc.transpose(0, 1, 3, 2, 4).reshape(C, SLAB, W)
        out[:, SLAB * core : SLAB * (core + 1), :] = yc.astype(np.float32)
    return out


if __name__ == "__main__":
    # native compile smoke (no hardware)
    import tempfile
    from concourse.bass_utils import compile_bass_kernel

    nc = _build_nc()
    with tempfile.TemporaryDirectory() as td:
        neff = compile_bass_kernel(nc, td)
        print("COMPILED OK:", neff)
